# revision 1
# baseline (speedup 1.0000x reference)
"""GraphSmoothingLoss on 8 Trainium2 NeuronCores.

loss = mean_e || f[src_e] - f[dst_e] ||_2   over E=800000 edges, f: [50000, 96] fp32.

Strategy (edge sharding): each core gets E/8 = 100000 edges (padded to
102400 = 128 * 800) plus a full replica of the feature table. Per core:
  - per column of 128 edges, one indirect DMA per stream gathers the 128
    src rows / 128 dst rows (HW contract: one index per dest partition),
  - every G=8 columns, DVE computes diff = src - dst, ACT squares in-place,
    DVE reduces each 96-wide segment to a per-edge squared distance,
  - a final ACT Sqrt-with-accumulate produces per-partition sums of distances,
  - host sums the 8 x 128 partials and divides by E.
Pad edges use src=dst=0 so they contribute exactly 0.
"""

import numpy as np

import concourse.bass as bass
import concourse.mybir as mybir

# Problem constants (hardcoded per contract: kernel.py must be self-contained).
N_NODES = 50000
D_FEAT = 96
N_EDGES = 800000
N_CORES = 8

P = 128          # SBUF partitions
NT = 800         # edge columns per core -> 102400 edge slots
G = 8            # columns per compute group
E_CORE = N_EDGES // N_CORES          # 100000 real edges per core
E_PAD = P * NT                       # 102400 slots per core

_NC_CACHE = {}


def build_nc(n_nodes=N_NODES, d=D_FEAT, nt=NT, g=G):
    """Build the per-core Bass program (SPMD: same program on all cores)."""
    ng = nt // g
    nc = bass.Bass()
    features = nc.declare_dram_parameter(
        "features", [n_nodes, d], mybir.dt.float32, isOutput=False
    )
    # idx layout [P, 2*nt]: columns [0, nt) = src, [nt, 2*nt) = dst;
    # edge slot (t, p) lives at [p, t] / [p, nt + t].
    edge_idx = nc.declare_dram_parameter(
        "edge_idx", [P, 2 * nt], mybir.dt.int32, isOutput=False
    )
    partial = nc.declare_dram_parameter(
        "partial", [P, 1], mybir.dt.float32, isOutput=True
    )

    with (
        nc.sbuf_tensor([P, 2 * nt], mybir.dt.int32) as idx_sb,
        nc.sbuf_tensor([P, g * d], mybir.dt.float32) as gs0,
        nc.sbuf_tensor([P, g * d], mybir.dt.float32) as gs1,
        nc.sbuf_tensor([P, g * d], mybir.dt.float32) as gd0,
        nc.sbuf_tensor([P, g * d], mybir.dt.float32) as gd1,
        nc.sbuf_tensor([P, g * d], mybir.dt.float32) as diff0,
        nc.sbuf_tensor([P, g * d], mybir.dt.float32) as diff1,
        nc.sbuf_tensor([P, nt], mybir.dt.float32) as norms2,
        nc.sbuf_tensor([P, nt], mybir.dt.float32) as sqrt_scratch,
        nc.sbuf_tensor([P, 1], mybir.dt.float32) as partial_sb,
        nc.semaphore() as idx_sem,
        nc.semaphore() as dma_sem0,
        nc.semaphore() as dma_sem1,
        nc.semaphore() as out_sem,
        nc.semaphore() as sub_sem,
        nc.semaphore() as act_sem,
        nc.semaphore() as red_sem,
        nc.Block() as block,
    ):
        gatS = [gs0, gs1]
        gatD = [gd0, gd1]
        diff = [diff0, diff1]
        dma_sems = [dma_sem0, dma_sem1]
        INC_PER_GROUP = 16 * 2 * g   # 2g indirect DMAs per group, +16 each

        @block.sync
        def _(sync):
            sync.dma_start(out=idx_sb[:], in_=edge_idx[:]).then_inc(idx_sem, 16)

        @block.gpsimd
        def _(gpsimd):
            gpsimd.wait_ge(idx_sem, 16)
            for gi in range(ng):
                s = gi % 2
                if gi >= 2:
                    # gather slots are free once sub(gi-2) has consumed them;
                    # the sem wait also keeps the per-slot DMA sem monotone.
                    gpsimd.wait_ge(sub_sem, gi - 1)
                    gpsimd.wait_ge(dma_sems[s], INC_PER_GROUP * (gi // 2))
                for k in range(g):
                    col = gi * g + k
                    gpsimd.indirect_dma_start(
                        out=gatS[s][:, k * d : (k + 1) * d],
                        out_offset=None,
                        in_=features[:, :],
                        in_offset=bass.IndirectOffsetOnAxis(
                            ap=idx_sb[:, col : col + 1], axis=0
                        ),
                    ).then_inc(dma_sems[s], 16)
                    gpsimd.indirect_dma_start(
                        out=gatD[s][:, k * d : (k + 1) * d],
                        out_offset=None,
                        in_=features[:, :],
                        in_offset=bass.IndirectOffsetOnAxis(
                            ap=idx_sb[:, nt + col : nt + col + 1], axis=0
                        ),
                    ).then_inc(dma_sems[s], 16)
            # output writeback
            gpsimd.wait_ge(act_sem, ng + 1)
            gpsimd.dma_start(out=partial[:], in_=partial_sb[:]).then_inc(out_sem, 16)
            gpsimd.wait_ge(out_sem, 16)

        @block.vector
        def _(vector):
            # software-pipelined order: sub(0), sub(1), red(0), sub(2), red(1), ...
            def sub(gi):
                s = gi % 2
                vector.wait_ge(dma_sems[s], INC_PER_GROUP * (gi // 2 + 1))
                nc.vector.tensor_tensor(
                    out=diff[s][:],
                    in0=gatS[s][:],
                    in1=gatD[s][:],
                    op=mybir.AluOpType.subtract,
                ).then_inc(sub_sem, 1)

            def red(gi):
                vector.wait_ge(act_sem, gi + 1)
                nc.vector.tensor_reduce(
                    out=norms2[:, gi * g : (gi + 1) * g],
                    in_=diff[gi % 2][:].rearrange("p (g d) -> p g d", d=d),
                    axis=mybir.AxisListType.X,
                    op=mybir.AluOpType.add,
                ).then_inc(red_sem, 1)

            sub(0)
            for gi in range(1, ng):
                sub(gi)
                red(gi - 1)
            red(ng - 1)

        @block.scalar
        def _(scalar):
            for gi in range(ng):
                scalar.wait_ge(sub_sem, gi + 1)
                nc.scalar.activation(
                    out=diff[gi % 2][:],
                    in_=diff[gi % 2][:],
                    func=mybir.ActivationFunctionType.Square,
                ).then_inc(act_sem, 1)
            # tail: per-edge sqrt, accumulate per-partition sum of distances
            scalar.wait_ge(red_sem, ng)
            nc.scalar.activation(
                out=sqrt_scratch[:],
                in_=norms2[:],
                func=mybir.ActivationFunctionType.Sqrt,
                accum_out=partial_sb[:],
            ).then_inc(act_sem, 1)

    return nc


def _pack_indices(src, dst, nt=NT):
    """[E_pad] int32 src/dst -> [P, 2*nt] int32: edge slot (t, p) -> [p, t]."""
    out = np.empty((P, 2 * nt), dtype=np.int32)
    out[:, :nt] = src.reshape(nt, P).T
    out[:, nt:] = dst.reshape(nt, P).T
    return out


def kernel(features, edge_index):
    from concourse.bass_utils import run_bass_kernel_spmd

    features = np.ascontiguousarray(np.asarray(features, dtype=np.float32))
    edge_index = np.asarray(edge_index)
    src = np.asarray(edge_index[0], dtype=np.int32)
    dst = np.asarray(edge_index[1], dtype=np.int32)

    key = ("main", NT, G)
    if key not in _NC_CACHE:
        _NC_CACHE[key] = build_nc()
    nc = _NC_CACHE[key]

    in_maps = []
    for c in range(N_CORES):
        s = np.zeros(E_PAD, dtype=np.int32)
        t = np.zeros(E_PAD, dtype=np.int32)
        s[:E_CORE] = src[c * E_CORE : (c + 1) * E_CORE]
        t[:E_CORE] = dst[c * E_CORE : (c + 1) * E_CORE]
        in_maps.append({"features": features, "edge_idx": _pack_indices(s, t)})

    res = run_bass_kernel_spmd(nc, in_maps, list(range(N_CORES)))
    total = np.float64(0.0)
    for c in range(N_CORES):
        total += np.asarray(res.results[c]["partial"], dtype=np.float64).sum()
    return np.float32(total / N_EDGES)



# revision 2
# speedup vs baseline: 1.0054x; 1.0054x over previous
"""GraphSmoothingLoss on 8 Trainium2 NeuronCores.

loss = mean_e || f[src_e] - f[dst_e] ||_2   over E=800000 edges, f: [50000, 96] fp32.

Strategy (edge sharding): each core gets E/8 = 100000 edges (padded to
102400 = 128 * 800) plus a full replica of the feature table. Per core:
  - per column of 128 edges, one indirect DMA per stream gathers the 128
    src rows / 128 dst rows (HW contract: one index per dest partition);
    784 columns instead of 800 trims 32 pure-padding gather instructions
    (the Pool SWDGE instruction rate, ~0.9 us each, is the binding
    constraint on this runtime - descriptor generation dominates),
  - every G=8 columns, DVE computes diff = src - dst, ACT squares in-place,
    DVE reduces each 96-wide segment to a per-edge squared distance,
  - a final ACT Sqrt-with-accumulate produces per-partition sums of distances,
  - host sums the 8 x 128 partials and divides by E.
Pad edges use src=dst=0 so they contribute exactly 0.
"""

import numpy as np

import concourse.bass as bass
import concourse.mybir as mybir

# Problem constants (hardcoded per contract: kernel.py must be self-contained).
N_NODES = 50000
D_FEAT = 96
N_EDGES = 800000
N_CORES = 8

P = 128          # SBUF partitions
NT = 784         # edge columns per core -> 100352 edge slots (min 782 for 100000 edges; 784 = 98 groups of G=8)
G = 8            # columns per compute group
E_CORE = N_EDGES // N_CORES          # 100000 real edges per core
E_PAD = P * NT                       # 102400 slots per core

_NC_CACHE = {}


def build_nc(n_nodes=N_NODES, d=D_FEAT, nt=NT, g=G):
    """Build the per-core Bass program (SPMD: same program on all cores)."""
    ng = nt // g
    nc = bass.Bass()
    features = nc.declare_dram_parameter(
        "features", [n_nodes, d], mybir.dt.float32, isOutput=False
    )
    # idx layout [P, 2*nt]: columns [0, nt) = src, [nt, 2*nt) = dst;
    # edge slot (t, p) lives at [p, t] / [p, nt + t].
    edge_idx = nc.declare_dram_parameter(
        "edge_idx", [P, 2 * nt], mybir.dt.int32, isOutput=False
    )
    partial = nc.declare_dram_parameter(
        "partial", [P, 1], mybir.dt.float32, isOutput=True
    )

    with (
        nc.sbuf_tensor([P, 2 * nt], mybir.dt.int32) as idx_sb,
        nc.sbuf_tensor([P, g * d], mybir.dt.float32) as gs0,
        nc.sbuf_tensor([P, g * d], mybir.dt.float32) as gs1,
        nc.sbuf_tensor([P, g * d], mybir.dt.float32) as gd0,
        nc.sbuf_tensor([P, g * d], mybir.dt.float32) as gd1,
        nc.sbuf_tensor([P, g * d], mybir.dt.float32) as diff0,
        nc.sbuf_tensor([P, g * d], mybir.dt.float32) as diff1,
        nc.sbuf_tensor([P, nt], mybir.dt.float32) as norms2,
        nc.sbuf_tensor([P, nt], mybir.dt.float32) as sqrt_scratch,
        nc.sbuf_tensor([P, 1], mybir.dt.float32) as partial_sb,
        nc.semaphore() as idx_sem,
        nc.semaphore() as dma_sem0,
        nc.semaphore() as dma_sem1,
        nc.semaphore() as out_sem,
        nc.semaphore() as sub_sem,
        nc.semaphore() as act_sem,
        nc.semaphore() as red_sem,
        nc.Block() as block,
    ):
        gatS = [gs0, gs1]
        gatD = [gd0, gd1]
        diff = [diff0, diff1]
        dma_sems = [dma_sem0, dma_sem1]
        INC_PER_GROUP = 16 * 2 * g   # 2g indirect DMAs per group, +16 each

        @block.sync
        def _(sync):
            sync.dma_start(out=idx_sb[:], in_=edge_idx[:]).then_inc(idx_sem, 16)

        @block.gpsimd
        def _(gpsimd):
            gpsimd.wait_ge(idx_sem, 16)
            for gi in range(ng):
                s = gi % 2
                if gi >= 2:
                    # gather slots are free once sub(gi-2) has consumed them;
                    # the sem wait also keeps the per-slot DMA sem monotone.
                    gpsimd.wait_ge(sub_sem, gi - 1)
                    gpsimd.wait_ge(dma_sems[s], INC_PER_GROUP * (gi // 2))
                for k in range(g):
                    col = gi * g + k
                    gpsimd.indirect_dma_start(
                        out=gatS[s][:, k * d : (k + 1) * d],
                        out_offset=None,
                        in_=features[:, :],
                        in_offset=bass.IndirectOffsetOnAxis(
                            ap=idx_sb[:, col : col + 1], axis=0
                        ),
                    ).then_inc(dma_sems[s], 16)
                    gpsimd.indirect_dma_start(
                        out=gatD[s][:, k * d : (k + 1) * d],
                        out_offset=None,
                        in_=features[:, :],
                        in_offset=bass.IndirectOffsetOnAxis(
                            ap=idx_sb[:, nt + col : nt + col + 1], axis=0
                        ),
                    ).then_inc(dma_sems[s], 16)
            # output writeback
            gpsimd.wait_ge(act_sem, ng + 1)
            gpsimd.dma_start(out=partial[:], in_=partial_sb[:]).then_inc(out_sem, 16)
            gpsimd.wait_ge(out_sem, 16)

        @block.vector
        def _(vector):
            # software-pipelined order: sub(0), sub(1), red(0), sub(2), red(1), ...
            def sub(gi):
                s = gi % 2
                vector.wait_ge(dma_sems[s], INC_PER_GROUP * (gi // 2 + 1))
                nc.vector.tensor_tensor(
                    out=diff[s][:],
                    in0=gatS[s][:],
                    in1=gatD[s][:],
                    op=mybir.AluOpType.subtract,
                ).then_inc(sub_sem, 1)

            def red(gi):
                vector.wait_ge(act_sem, gi + 1)
                nc.vector.tensor_reduce(
                    out=norms2[:, gi * g : (gi + 1) * g],
                    in_=diff[gi % 2][:].rearrange("p (g d) -> p g d", d=d),
                    axis=mybir.AxisListType.X,
                    op=mybir.AluOpType.add,
                ).then_inc(red_sem, 1)

            sub(0)
            for gi in range(1, ng):
                sub(gi)
                red(gi - 1)
            red(ng - 1)

        @block.scalar
        def _(scalar):
            for gi in range(ng):
                scalar.wait_ge(sub_sem, gi + 1)
                nc.scalar.activation(
                    out=diff[gi % 2][:],
                    in_=diff[gi % 2][:],
                    func=mybir.ActivationFunctionType.Square,
                ).then_inc(act_sem, 1)
            # tail: per-edge sqrt, accumulate per-partition sum of distances
            scalar.wait_ge(red_sem, ng)
            nc.scalar.activation(
                out=sqrt_scratch[:],
                in_=norms2[:],
                func=mybir.ActivationFunctionType.Sqrt,
                accum_out=partial_sb[:],
            ).then_inc(act_sem, 1)

    return nc


def _pack_indices(src, dst, nt=NT):
    """[E_pad] int32 src/dst -> [P, 2*nt] int32: edge slot (t, p) -> [p, t]."""
    out = np.empty((P, 2 * nt), dtype=np.int32)
    out[:, :nt] = src.reshape(nt, P).T
    out[:, nt:] = dst.reshape(nt, P).T
    return out


def kernel(features, edge_index):
    from concourse.bass_utils import run_bass_kernel_spmd

    features = np.ascontiguousarray(np.asarray(features, dtype=np.float32))
    edge_index = np.asarray(edge_index)
    src = np.asarray(edge_index[0], dtype=np.int32)
    dst = np.asarray(edge_index[1], dtype=np.int32)

    key = ("main", NT, G)
    if key not in _NC_CACHE:
        _NC_CACHE[key] = build_nc()
    nc = _NC_CACHE[key]

    in_maps = []
    for c in range(N_CORES):
        s = np.zeros(E_PAD, dtype=np.int32)
        t = np.zeros(E_PAD, dtype=np.int32)
        s[:E_CORE] = src[c * E_CORE : (c + 1) * E_CORE]
        t[:E_CORE] = dst[c * E_CORE : (c + 1) * E_CORE]
        in_maps.append({"features": features, "edge_idx": _pack_indices(s, t)})

    res = run_bass_kernel_spmd(nc, in_maps, list(range(N_CORES)))
    total = np.float64(0.0)
    for c in range(N_CORES):
        total += np.asarray(res.results[c]["partial"], dtype=np.float64).sum()
    return np.float32(total / N_EDGES)

